# revision 22
# baseline (speedup 1.0000x reference)
"""Trainium2 Bass kernel for BinaryHead: logits = (l2norm(fea) @ W.T + b) * 16.

Sharding: data-parallel over the batch dim across 8 NeuronCores (2048 rows
each).  The host stages each core's shard transposed and pre-swizzled into
the SBUF tile layout [128 part, 4 chunk, 16 panel, 512 batch] (bf16), so the
embedding dim lands on partitions and every DMA descriptor is a 4 KB flat
copy.  The whole 8 MB shard is SBUF-resident.

v3 structure - batch-chunk-outer streaming:
  The batch is split into four 512-col chunks (PSUM bank width).  The HBM
  stream delivers chunk 0's sixteen panels first, then chunk 1, ..., so
  chunk j's sumsq/z accumulation completes at ~(j+1)/4 of the stream and
  its epilogue (rsqrt via Ln/Exp + class-broadcast matmul + scale + bias)
  hides INSIDE the stream; only the final chunk's epilogue is exposed.
  Chunk 3's first half is delivered mid-stream so the tail only carries
  its last panels.

  Squares (x2, for the sumsq reduction) run on ACT (fp8 out -> DoubleRow
  sumsq matmuls), DVE (mix of fp8 and 2x-rate bf16 -> plain bf16 sumsq
  matmuls) and GpSimd (2 mid-stream groups, fp8).  DMA issue is spread
  over sync/scalar-HWDGE + gpsimd-SWDGE with <=10 issues per engine so
  semaphore-reuse stalls never block a compute engine.

  ~48 dummy matmuls from a memset tile keep the PE HAM activity window
  busy from the end of the framework preamble so real matmuls run warm.
"""

from contextlib import ExitStack

import numpy as np

NUM_CLASS = 4
EMB = 2048
BATCH = 16384
N_CORES = 8
ROWS = BATCH // N_CORES  # 2048 rows per core
S = 16.0

N_ETILES = EMB // 128  # 16 e-panels per core
N_BCHUNK = ROWS // 512  # 4 psum-width chunks of the batch

# stream/processing order: (chunk j, first panel, n panels, sq engine, fp8?)
# A=ACT, V=DVE, G=GpSimd.  Chunk 3's first half is delivered mid-stream so
# the tail only carries its last 6 panels (pairs + singles).
ORDER = [
    (0, 0, 2, "V", False),
    (0, 2, 2, "A", True),
    (0, 4, 4, "A", True),
    (0, 8, 4, "V", True),
    (0, 12, 4, "G", True),
    (1, 0, 4, "G", True),
    (1, 4, 4, "A", True),
    (1, 8, 4, "V", True),
    (1, 12, 4, "G", True),
    (3, 0, 4, "A", True),
    (3, 4, 4, "V", True),
    (2, 0, 4, "G", True),
    (2, 4, 4, "A", True),
    (2, 8, 4, "V", False),
    (2, 12, 4, "A", True),
    (3, 8, 2, "V", False),
    (3, 10, 2, "A", True),
    (3, 12, 2, "A", True),
    (3, 14, 1, "V", False),
    (3, 15, 1, "V", False),
]
# ring per group index: s=sync, g=gpsimd, c=scalar (each <=10 issues)
RINGS = "sgcsgcsgsgsgsgsgsgsg"
assert len(RINGS) == len(ORDER)

_CACHE = {}


def _build_nc():
    import concourse.bacc as bacc
    import concourse.mybir as mybir
    import concourse.tile as tile
    from concourse.hw_specs import get_activation_tables

    f32 = mybir.dt.float32
    f32r = mybir.dt.float32r
    bf16 = mybir.dt.bfloat16
    fp8 = mybir.dt.float8e4

    nc = bacc.Bacc(
        "TRN2",
        target_bir_lowering=False,
        debug=False,
        enable_asserts=False,
        num_devices=N_CORES,
    )

    # feaS[p, j, t, c] = bf16(fea_shard[512j+c, 128t+p])
    feaS = nc.dram_tensor(
        "feaS", [128, N_BCHUNK, N_ETILES, 512], bf16, kind="ExternalInput"
    ).ap()
    wt = nc.dram_tensor(
        "wt", [128, N_ETILES * NUM_CLASS], bf16, kind="ExternalInput"
    ).ap()
    onesv = nc.dram_tensor("onesv", [128, 2, 16], fp8, kind="ExternalInput").ap()
    sones = nc.dram_tensor("sones", [1, NUM_CLASS], f32r, kind="ExternalInput").ap()
    sbias = nc.dram_tensor("sbias", [NUM_CLASS, 1], f32, kind="ExternalInput").ap()
    outT = nc.dram_tensor("outT", [NUM_CLASS, ROWS], f32, kind="ExternalOutput").ap()

    ring_of = {"s": None, "g": None, "c": None}  # filled after nc exists
    with tile.TileContext(nc) as tc, ExitStack() as ctx:
        ring_of = {"s": nc.sync, "g": nc.gpsimd, "c": nc.scalar}
        pconst = ctx.enter_context(tc.tile_pool(name="pconst", bufs=1))
        pdata = ctx.enter_context(tc.tile_pool(name="pdata", bufs=1))
        psq = ctx.enter_context(tc.tile_pool(name="psq", bufs=1))
        pep = ctx.enter_context(tc.tile_pool(name="pep", bufs=1))
        pz = ctx.enter_context(tc.tile_pool(name="pz", bufs=1, space="PSUM"))
        ps = ctx.enter_context(tc.tile_pool(name="ps", bufs=1, space="PSUM"))

        # ACT table preload: set with Square + Ln + Exp (no mid-kernel switch)
        nlx_id = list(get_activation_tables(nc.m.arch)).index(
            "natural_log_exp_and_others"
        )
        nc.scalar.add_instruction(
            mybir.InstLoadActFuncSet(name=f"I-{nc.next_id()}", act_func_set_id=nlx_id)
        )

        # ---- constants -------------------------------------------------
        # ring-entry completions serialize (~2us receipt each), so at most
        # ONE tiny const rides ahead of the first data entry per ring; the
        # epilogue-only consts go on the lightly-used scalar ring
        wt_s = pconst.tile([128, N_ETILES * NUM_CLASS], bf16)
        nc.sync.dma_start(out=wt_s, in_=wt)
        ones_s = pconst.tile([128, 2, 16], fp8)
        nc.gpsimd.dma_start(out=ones_s, in_=onesv)
        sones_s = pconst.tile([1, NUM_CLASS], f32r)
        nc.scalar.dma_start(out=sones_s, in_=sones)
        sbias_s = pconst.tile([NUM_CLASS, 1], f32)
        nc.scalar.dma_start(out=sbias_s, in_=sbias)
        onesbf_s = pconst.tile([128, 1], bf16)
        nc.vector.memset(onesbf_s, 1.0)
        zero1_s = pconst.tile([1, 1], f32)
        nc.vector.memset(zero1_s, 0.0)
        zero128_s = pconst.tile([128, 1], f32)
        nc.vector.memset(zero128_s, 0.0)
        lnS_s = pconst.tile([1, 1], f32)
        nc.vector.memset(lnS_s, float(np.log(S)))
        dummy_s = pconst.tile([128, 64], bf16)
        nc.vector.memset(dummy_s, 0.0)

        # ---- resident tiles (j-major free layout) ----------------------
        xt = pdata.tile([128, N_BCHUNK, N_ETILES, 512], bf16)  # 64 KB/part
        x2f = psq.tile([128, N_BCHUNK, N_ETILES, 512], fp8, name="x2f")
        x2b = psq.tile([128, N_BCHUNK, N_ETILES, 512], bf16, name="x2b")

        zt_ps = pz.tile([NUM_CLASS, ROWS], f32, tag="zt")  # 4 banks
        ss_ps = [
            ps.tile([1, 512], f32, tag="ssrnb", bufs=4, name=f"ss{j}")
            for j in range(N_BCHUNK)
        ]
        rnb = [
            ps.tile([NUM_CLASS, 512], f32, tag="ssrnb", bufs=4, name=f"rnb{j}")
            for j in range(N_BCHUNK)
        ]
        z_s = pep.tile([NUM_CLASS, ROWS], f32)
        lnss_s = pep.tile([1, ROWS], f32)
        rnorm_s = pep.tile([1, ROWS], f32r)
        out_s = pep.tile([NUM_CLASS, ROWS], f32)
        outb_s = pep.tile([NUM_CLASS, ROWS], f32)

        # ---- PE warm-up (no data deps) ---------------------------------
        for _ in range(72):
            nc.tensor.matmul(
                zt_ps[:, 0:64],
                dummy_s[:, 0:NUM_CLASS],
                dummy_s[:, 0:64],
                start=True,
                stop=True,
            )

        # ---- all data DMAs up front, in stream order -------------------
        for gi, (j, t0, np_, eng, is8) in enumerate(ORDER):
            ring = ring_of[RINGS[gi]]
            ring.dma_start(
                out=xt[:, j, t0 : t0 + np_, :], in_=feaS[:, j, t0 : t0 + np_, :]
            )

        # ---- compute helpers -------------------------------------------
        def square(j, t0, np_, eng, is8):
            dst = x2f if is8 else x2b
            src = xt[:, j, t0 : t0 + np_, :]
            if eng == "A":
                nc.scalar.activation(
                    out=dst[:, j, t0 : t0 + np_, :],
                    in_=src,
                    func=mybir.ActivationFunctionType.Square,
                    bias=zero128_s,
                    scale=1.0,
                )
            elif eng == "G":
                nc.gpsimd.tensor_mul(dst[:, j, t0 : t0 + np_, :], src, src)
            else:
                nc.vector.tensor_mul(dst[:, j, t0 : t0 + np_, :], src, src)

        z_seen = [0] * N_BCHUNK

        def z_mm(t, j):
            z_seen[j] += 1
            nc.tensor.matmul(
                zt_ps[:, j * 512 : (j + 1) * 512],
                wt_s[:, t * NUM_CLASS : (t + 1) * NUM_CLASS],
                xt[:, j, t, :],
                start=(z_seen[j] == 1),
                stop=(z_seen[j] == N_ETILES),
            )

        ss_seen = [0] * N_BCHUNK

        def ss_mm(j, pair, is8):
            # pair k covers panels (2k, 2k+1); fp8 pairs use one DoubleRow
            # MM, bf16 pairs two plain MMs (counted as one "pair")
            ss_seen[j] += 1
            start = ss_seen[j] == 1
            stop = ss_seen[j] == N_ETILES // 2
            if is8:
                nc.tensor.matmul(
                    ss_ps[j],
                    ones_s[:, :, 0:1],
                    x2f[:, j, 2 * pair : 2 * pair + 2, :],
                    perf_mode=mybir.MatmulPerfMode.DoubleRow,
                    start=start,
                    stop=stop,
                )
            else:
                nc.tensor.matmul(
                    ss_ps[j], onesbf_s, x2b[:, j, 2 * pair, :], start=start, stop=False
                )
                nc.tensor.matmul(
                    ss_ps[j],
                    onesbf_s,
                    x2b[:, j, 2 * pair + 1, :],
                    start=False,
                    stop=stop,
                )

        def epilogue_pre(j):
            # non-PE part: runs as soon as chunk j's accumulation stops
            bsl = slice(j * 512, (j + 1) * 512)
            nc.vector.tensor_copy(z_s[:, bsl], zt_ps[:, bsl])
            # S/sqrt(ss) = exp(-0.5*ln(ss) + ln(S))
            nc.scalar.activation(
                out=lnss_s[:, bsl],
                in_=ss_ps[j],
                func=mybir.ActivationFunctionType.Ln,
                bias=zero1_s,
                scale=1.0,
            )
            nc.scalar.activation(
                out=rnorm_s[:, bsl],
                in_=lnss_s[:, bsl],
                func=mybir.ActivationFunctionType.Exp,
                bias=lnS_s,
                scale=-0.5,
            )

        def epilogue_post(j):
            # PE broadcast + final scale/bias + store; deferred one group so
            # the rnb matmul never head-of-line blocks the PE on ACT's Exp
            bsl = slice(j * 512, (j + 1) * 512)
            nc.tensor.matmul(rnb[j], sones_s, rnorm_s[:, bsl], start=True, stop=True)
            nc.vector.tensor_mul(out_s[:, bsl], z_s[:, bsl], rnb[j])
            nc.vector.tensor_scalar_add(
                outb_s[:, bsl], in0=out_s[:, bsl], scalar1=sbias_s
            )
            nc.scalar.dma_start(out=outT[:, bsl], in_=outb_s[:, bsl])

        # ---- main pipeline ---------------------------------------------
        # squares + z MMs in stream order; ss pair-MMs lag their group by
        # LAG entries (grace for the square), except the tail where they
        # chain directly; a chunk's epilogue fires once its 8 pairs are in.
        LAG = 3
        pending = []  # (j, pair, is8) awaiting ss issue
        deferred_post = []  # epilogue_post(j) waiting one group
        ss_remaining = [N_ETILES // 2] * N_BCHUNK

        def flush_ss(n):
            for _ in range(min(n, len(pending))):
                j, pair, is8 = pending.pop(0)
                ss_mm(j, pair, is8)
                ss_remaining[j] -= 1
                if ss_remaining[j] == 0:
                    epilogue_pre(j)
                    deferred_post.append(j)

        group_pairs_done = 0
        for gi, (j, t0, np_, eng, is8) in enumerate(ORDER):
            square(j, t0, np_, eng, is8)
            for t in range(t0, t0 + np_):
                z_mm(t, j)
            while deferred_post:
                epilogue_post(deferred_post.pop(0))
            if np_ == 1:
                # singles: pair forms only when the second panel arrives
                if t0 % 2 == 1:
                    pending.append((j, t0 // 2, is8))
            else:
                for pair in range(t0 // 2, (t0 + np_) // 2):
                    pending.append((j, pair, is8))
            group_pairs_done += 1
            if gi >= len(ORDER) - 5:
                flush_ss(len(pending))  # tail: chain directly
            elif group_pairs_done > LAG:
                flush_ss(ORDER[gi - LAG][2] // 2 or 1)
        flush_ss(len(pending))
        while deferred_post:
            epilogue_post(deferred_post.pop(0))

    nc.compile()
    return nc


def _get_nc():
    if "nc" not in _CACHE:
        _CACHE["nc"] = _build_nc()
    return _CACHE["nc"]


def _stage_inputs(fea, W, b):
    import ml_dtypes

    fea = np.asarray(fea, dtype=np.float32)
    W = np.asarray(W, dtype=np.float32)
    b = np.asarray(b, dtype=np.float32)

    # wt[p, 4t+c] = W[c, 128t+p]
    wt = np.ascontiguousarray(
        W.reshape(NUM_CLASS, N_ETILES, 128).transpose(2, 1, 0).reshape(128, -1)
    ).astype(ml_dtypes.bfloat16)
    onesv = np.zeros((128, 2, 16), dtype=ml_dtypes.float8_e4m3)
    onesv[:, :, 0] = 1.0
    sones = np.ones((1, NUM_CLASS), dtype=np.float32)
    sbias = (S * b).reshape(NUM_CLASS, 1).astype(np.float32)

    in_maps = []
    for i in range(N_CORES):
        shard = fea[i * ROWS : (i + 1) * ROWS, :]
        # feaS[p, j, t, c] = shard[512j+c, 128t+p]
        feaS = np.ascontiguousarray(
            shard.T.reshape(N_ETILES, 128, N_BCHUNK, 512).transpose(1, 2, 0, 3)
        ).astype(ml_dtypes.bfloat16)
        in_maps.append(
            {"feaS": feaS, "wt": wt, "onesv": onesv, "sones": sones, "sbias": sbias}
        )
    return in_maps


def run(fea, W, b, trace=False):
    from concourse.bass_utils import run_bass_kernel_spmd

    nc = _get_nc()
    in_maps = _stage_inputs(fea, W, b)
    res = run_bass_kernel_spmd(nc, in_maps, core_ids=list(range(N_CORES)), trace=trace)
    out = np.empty((BATCH, NUM_CLASS), dtype=np.float32)
    for i in range(N_CORES):
        out[i * ROWS : (i + 1) * ROWS, :] = res.results[i]["outT"].T
    return out, res


def kernel(fea, W, b):
    out, _ = run(fea, W, b, trace=False)
    return out


# revision 24
# speedup vs baseline: 1.0197x; 1.0197x over previous
"""Trainium2 Bass kernel for BinaryHead: logits = (l2norm(fea) @ W.T + b) * 16.

Sharding: data-parallel over the batch dim across 8 NeuronCores (2048 rows
each).  The host stages each core's shard transposed and pre-swizzled into
the SBUF tile layout [128 part, 4 chunk, 16 panel, 512 batch] (bf16), so the
embedding dim lands on partitions and every DMA descriptor is a 4 KB flat
copy.  The whole 8 MB shard is SBUF-resident.

v3 structure - batch-chunk-outer streaming:
  The batch is split into four 512-col chunks (PSUM bank width).  The HBM
  stream delivers chunk 0's sixteen panels first, then chunk 1, ..., so
  chunk j's sumsq/z accumulation completes at ~(j+1)/4 of the stream and
  its epilogue (rsqrt via Ln/Exp + class-broadcast matmul + scale + bias)
  hides INSIDE the stream; only the final chunk's epilogue is exposed.
  Chunk 3's first half is delivered mid-stream so the tail only carries
  its last panels.

  Squares (x2, for the sumsq reduction) run on ACT (fp8 out -> DoubleRow
  sumsq matmuls), DVE (mix of fp8 and 2x-rate bf16 -> plain bf16 sumsq
  matmuls) and GpSimd (2 mid-stream groups, fp8).  DMA issue is spread
  over sync/scalar-HWDGE + gpsimd-SWDGE with <=10 issues per engine so
  semaphore-reuse stalls never block a compute engine.

  ~48 dummy matmuls from a memset tile keep the PE HAM activity window
  busy from the end of the framework preamble so real matmuls run warm.
"""

from contextlib import ExitStack

import numpy as np

NUM_CLASS = 4
EMB = 2048
BATCH = 16384
N_CORES = 8
ROWS = BATCH // N_CORES  # 2048 rows per core
S = 16.0

N_ETILES = EMB // 128  # 16 e-panels per core
N_BCHUNK = ROWS // 512  # 4 psum-width chunks of the batch

# stream/processing order: (chunk j, first panel, n panels, sq engine, fp8?)
# A=ACT, V=DVE, G=GpSimd.  Chunk 3's first half is delivered mid-stream so
# the tail only carries its last 6 panels (pairs + singles).
# DMA entries: (ring, chunk j, first panel, n panels).  >=1MB entries
# amortize the ~2.2us per-ring-entry overhead (measured: 512KB entries cap
# a ring at ~135 B/ns; 1MB -> ~224, two rings together saturate HBM).
# <=7 entries per ring so issue instructions never stall on semaphore
# reuse.  Chunk 3's first half rides mid-stream; its tail is fine-grained.
ENTRIES = [
    ("s", 0, 0, 8),
    ("g", 0, 8, 8),
    ("s", 1, 8, 8),
    ("g", 1, 0, 8),
    ("s", 3, 0, 8),
    ("g", 2, 0, 8),
    ("s", 2, 8, 8),
    ("g", 3, 8, 2),
    ("s", 3, 10, 2),
    ("g", 3, 12, 2),
    ("s", 3, 14, 1),
    ("g", 3, 15, 1),
]
# compute groups in nominal arrival order: (chunk j, first panel, n panels,
# sq engine, fp8?)
ORDER = [
    (0, 0, 4, "A", True),
    (0, 4, 4, "V", False),
    (0, 8, 4, "G", True),
    (0, 12, 4, "A", True),
    (1, 8, 4, "V", True),
    (1, 12, 4, "V", False),
    (1, 0, 4, "G", True),
    (1, 4, 4, "A", True),
    (3, 0, 4, "A", True),
    (3, 4, 4, "V", False),
    (2, 0, 4, "A", True),
    (2, 4, 4, "V", True),
    (2, 8, 4, "A", True),
    (2, 12, 4, "V", True),
    (3, 8, 2, "V", False),
    (3, 10, 2, "A", True),
    (3, 12, 2, "V", False),
    (3, 14, 1, "V", False),
    (3, 15, 1, "V", False),
]

_CACHE = {}


def _build_nc():
    import concourse.bacc as bacc
    import concourse.mybir as mybir
    import concourse.tile as tile
    from concourse.hw_specs import get_activation_tables

    f32 = mybir.dt.float32
    f32r = mybir.dt.float32r
    bf16 = mybir.dt.bfloat16
    fp8 = mybir.dt.float8e4

    nc = bacc.Bacc(
        "TRN2",
        target_bir_lowering=False,
        debug=False,
        enable_asserts=False,
        num_devices=N_CORES,
    )

    # feaS[p, j, t, c] = bf16(fea_shard[512j+c, 128t+p])
    feaS = nc.dram_tensor(
        "feaS", [128, N_BCHUNK, N_ETILES, 512], bf16, kind="ExternalInput"
    ).ap()
    wt = nc.dram_tensor(
        "wt", [128, N_ETILES * NUM_CLASS], bf16, kind="ExternalInput"
    ).ap()
    onesv = nc.dram_tensor("onesv", [128, 2, 16], fp8, kind="ExternalInput").ap()
    sones = nc.dram_tensor("sones", [1, NUM_CLASS], f32r, kind="ExternalInput").ap()
    sbias = nc.dram_tensor("sbias", [NUM_CLASS, 1], f32, kind="ExternalInput").ap()
    outT = nc.dram_tensor("outT", [NUM_CLASS, ROWS], f32, kind="ExternalOutput").ap()

    ring_of = {"s": None, "g": None, "c": None}  # filled after nc exists
    with tile.TileContext(nc) as tc, ExitStack() as ctx:
        ring_of = {"s": nc.sync, "g": nc.gpsimd, "c": nc.scalar}
        pconst = ctx.enter_context(tc.tile_pool(name="pconst", bufs=1))
        pdata = ctx.enter_context(tc.tile_pool(name="pdata", bufs=1))
        psq = ctx.enter_context(tc.tile_pool(name="psq", bufs=1))
        pep = ctx.enter_context(tc.tile_pool(name="pep", bufs=1))
        pz = ctx.enter_context(tc.tile_pool(name="pz", bufs=1, space="PSUM"))
        ps = ctx.enter_context(tc.tile_pool(name="ps", bufs=1, space="PSUM"))

        # ACT table preload: set with Square + Ln + Exp (no mid-kernel switch)
        nlx_id = list(get_activation_tables(nc.m.arch)).index(
            "natural_log_exp_and_others"
        )
        nc.scalar.add_instruction(
            mybir.InstLoadActFuncSet(name=f"I-{nc.next_id()}", act_func_set_id=nlx_id)
        )

        # ---- constants -------------------------------------------------
        # ring-entry completions serialize (~2us receipt each), so at most
        # ONE tiny const rides ahead of the first data entry per ring; the
        # epilogue-only consts go on the lightly-used scalar ring
        wt_s = pconst.tile([128, N_ETILES * NUM_CLASS], bf16)
        nc.sync.dma_start(out=wt_s, in_=wt)
        ones_s = pconst.tile([128, 2, 16], fp8)
        nc.gpsimd.dma_start(out=ones_s, in_=onesv)
        sones_s = pconst.tile([1, NUM_CLASS], f32r)
        nc.scalar.dma_start(out=sones_s, in_=sones)
        sbias_s = pconst.tile([NUM_CLASS, 1], f32)
        nc.scalar.dma_start(out=sbias_s, in_=sbias)
        onesbf_s = pconst.tile([128, 1], bf16)
        nc.vector.memset(onesbf_s, 1.0)
        zero1_s = pconst.tile([1, 1], f32)
        nc.vector.memset(zero1_s, 0.0)
        zero128_s = pconst.tile([128, 1], f32)
        nc.vector.memset(zero128_s, 0.0)
        lnS_s = pconst.tile([1, 1], f32)
        nc.vector.memset(lnS_s, float(np.log(S)))
        dummy_s = pconst.tile([128, 64], bf16)
        nc.vector.memset(dummy_s, 0.0)

        # ---- resident tiles (j-major free layout) ----------------------
        xt = pdata.tile([128, N_BCHUNK, N_ETILES, 512], bf16)  # 64 KB/part
        x2f = psq.tile([128, N_BCHUNK, N_ETILES, 512], fp8, name="x2f")
        x2b = psq.tile([128, N_BCHUNK, N_ETILES, 512], bf16, name="x2b")

        zt_ps = pz.tile([NUM_CLASS, ROWS], f32, tag="zt")  # 4 banks
        ss_ps = [
            ps.tile([1, 512], f32, tag="ssrnb", bufs=4, name=f"ss{j}")
            for j in range(N_BCHUNK)
        ]
        rnb = [
            ps.tile([NUM_CLASS, 512], f32, tag="ssrnb", bufs=4, name=f"rnb{j}")
            for j in range(N_BCHUNK)
        ]
        z_s = pep.tile([NUM_CLASS, ROWS], f32)
        lnss_s = pep.tile([1, ROWS], f32)
        rnorm_s = pep.tile([1, ROWS], f32r)
        out_s = pep.tile([NUM_CLASS, ROWS], f32)
        outb_s = pep.tile([NUM_CLASS, ROWS], f32)

        # ---- PE warm-up (no data deps) ---------------------------------
        for _ in range(72):
            nc.tensor.matmul(
                zt_ps[:, 0:64],
                dummy_s[:, 0:NUM_CLASS],
                dummy_s[:, 0:64],
                start=True,
                stop=True,
            )

        # ---- all data DMAs up front, in stream order -------------------
        for ring_key, j, t0, np_ in ENTRIES:
            ring = ring_of[ring_key]
            ring.dma_start(
                out=xt[:, j, t0 : t0 + np_, :], in_=feaS[:, j, t0 : t0 + np_, :]
            )

        # ---- compute helpers -------------------------------------------
        def square(j, t0, np_, eng, is8):
            dst = x2f if is8 else x2b
            src = xt[:, j, t0 : t0 + np_, :]
            if eng == "A":
                nc.scalar.activation(
                    out=dst[:, j, t0 : t0 + np_, :],
                    in_=src,
                    func=mybir.ActivationFunctionType.Square,
                    bias=zero128_s,
                    scale=1.0,
                )
            elif eng == "G":
                nc.gpsimd.tensor_mul(dst[:, j, t0 : t0 + np_, :], src, src)
            else:
                nc.vector.tensor_mul(dst[:, j, t0 : t0 + np_, :], src, src)

        z_seen = [0] * N_BCHUNK

        def z_mm(t, j):
            z_seen[j] += 1
            nc.tensor.matmul(
                zt_ps[:, j * 512 : (j + 1) * 512],
                wt_s[:, t * NUM_CLASS : (t + 1) * NUM_CLASS],
                xt[:, j, t, :],
                start=(z_seen[j] == 1),
                stop=(z_seen[j] == N_ETILES),
            )

        ss_seen = [0] * N_BCHUNK

        def ss_mm(j, pair, is8):
            # pair k covers panels (2k, 2k+1); fp8 pairs use one DoubleRow
            # MM, bf16 pairs two plain MMs (counted as one "pair")
            ss_seen[j] += 1
            start = ss_seen[j] == 1
            stop = ss_seen[j] == N_ETILES // 2
            if is8:
                nc.tensor.matmul(
                    ss_ps[j],
                    ones_s[:, :, 0:1],
                    x2f[:, j, 2 * pair : 2 * pair + 2, :],
                    perf_mode=mybir.MatmulPerfMode.DoubleRow,
                    start=start,
                    stop=stop,
                )
            else:
                nc.tensor.matmul(
                    ss_ps[j], onesbf_s, x2b[:, j, 2 * pair, :], start=start, stop=False
                )
                nc.tensor.matmul(
                    ss_ps[j],
                    onesbf_s,
                    x2b[:, j, 2 * pair + 1, :],
                    start=False,
                    stop=stop,
                )

        def epilogue_pre(j):
            # non-PE part: runs as soon as chunk j's accumulation stops
            bsl = slice(j * 512, (j + 1) * 512)
            nc.vector.tensor_copy(z_s[:, bsl], zt_ps[:, bsl])
            # S/sqrt(ss) = exp(-0.5*ln(ss) + ln(S))
            nc.scalar.activation(
                out=lnss_s[:, bsl],
                in_=ss_ps[j],
                func=mybir.ActivationFunctionType.Ln,
                bias=zero1_s,
                scale=1.0,
            )
            nc.scalar.activation(
                out=rnorm_s[:, bsl],
                in_=lnss_s[:, bsl],
                func=mybir.ActivationFunctionType.Exp,
                bias=lnS_s,
                scale=-0.5,
            )

        def epilogue_post(j):
            # PE broadcast + final scale/bias + store; deferred one group so
            # the rnb matmul never head-of-line blocks the PE on ACT's Exp
            bsl = slice(j * 512, (j + 1) * 512)
            nc.tensor.matmul(rnb[j], sones_s, rnorm_s[:, bsl], start=True, stop=True)
            nc.vector.tensor_mul(out_s[:, bsl], z_s[:, bsl], rnb[j])
            nc.vector.tensor_scalar_add(
                outb_s[:, bsl], in0=out_s[:, bsl], scalar1=sbias_s
            )
            nc.scalar.dma_start(out=outT[:, bsl], in_=outb_s[:, bsl])

        # ---- main pipeline ---------------------------------------------
        # squares + z MMs in stream order; ss pair-MMs lag their group by
        # LAG entries (grace for the square), except the tail where they
        # chain directly; a chunk's epilogue fires once its 8 pairs are in.
        LAG = 3
        pending = []  # (j, pair, is8) awaiting ss issue
        deferred_post = []  # epilogue_post(j) waiting one group
        ss_remaining = [N_ETILES // 2] * N_BCHUNK

        def flush_ss(n):
            for _ in range(min(n, len(pending))):
                j, pair, is8 = pending.pop(0)
                ss_mm(j, pair, is8)
                ss_remaining[j] -= 1
                if ss_remaining[j] == 0:
                    epilogue_pre(j)
                    deferred_post.append(j)

        group_pairs_done = 0
        for gi, (j, t0, np_, eng, is8) in enumerate(ORDER):
            square(j, t0, np_, eng, is8)
            for t in range(t0, t0 + np_):
                z_mm(t, j)
            while deferred_post:
                epilogue_post(deferred_post.pop(0))
            if np_ == 1:
                # singles: pair forms only when the second panel arrives
                if t0 % 2 == 1:
                    pending.append((j, t0 // 2, is8))
            else:
                for pair in range(t0 // 2, (t0 + np_) // 2):
                    pending.append((j, pair, is8))
            group_pairs_done += 1
            if gi >= len(ORDER) - 5:
                flush_ss(len(pending))  # tail: chain directly
            elif group_pairs_done > LAG:
                flush_ss(ORDER[gi - LAG][2] // 2 or 1)
        flush_ss(len(pending))
        while deferred_post:
            epilogue_post(deferred_post.pop(0))

    nc.compile()
    return nc


def _get_nc():
    if "nc" not in _CACHE:
        _CACHE["nc"] = _build_nc()
    return _CACHE["nc"]


def _stage_inputs(fea, W, b):
    import ml_dtypes

    fea = np.asarray(fea, dtype=np.float32)
    W = np.asarray(W, dtype=np.float32)
    b = np.asarray(b, dtype=np.float32)

    # wt[p, 4t+c] = W[c, 128t+p]
    wt = np.ascontiguousarray(
        W.reshape(NUM_CLASS, N_ETILES, 128).transpose(2, 1, 0).reshape(128, -1)
    ).astype(ml_dtypes.bfloat16)
    onesv = np.zeros((128, 2, 16), dtype=ml_dtypes.float8_e4m3)
    onesv[:, :, 0] = 1.0
    sones = np.ones((1, NUM_CLASS), dtype=np.float32)
    sbias = (S * b).reshape(NUM_CLASS, 1).astype(np.float32)

    in_maps = []
    for i in range(N_CORES):
        shard = fea[i * ROWS : (i + 1) * ROWS, :]
        # feaS[p, j, t, c] = shard[512j+c, 128t+p]
        feaS = np.ascontiguousarray(
            shard.T.reshape(N_ETILES, 128, N_BCHUNK, 512).transpose(1, 2, 0, 3)
        ).astype(ml_dtypes.bfloat16)
        in_maps.append(
            {"feaS": feaS, "wt": wt, "onesv": onesv, "sones": sones, "sbias": sbias}
        )
    return in_maps


def run(fea, W, b, trace=False):
    from concourse.bass_utils import run_bass_kernel_spmd

    nc = _get_nc()
    in_maps = _stage_inputs(fea, W, b)
    res = run_bass_kernel_spmd(nc, in_maps, core_ids=list(range(N_CORES)), trace=trace)
    out = np.empty((BATCH, NUM_CLASS), dtype=np.float32)
    for i in range(N_CORES):
        out[i * ROWS : (i + 1) * ROWS, :] = res.results[i]["outT"].T
    return out, res


def kernel(fea, W, b):
    out, _ = run(fea, W, b, trace=False)
    return out
